# revision 50
# baseline (speedup 1.0000x reference)
"""Trainium2 Bass kernel for nn_Attention_927712936452.

Two-branch attention (self branch over x, cross branch of y-queries over
concat(x,y) keys/values), QKV + output projection, H=12 heads of 64.

Distribution: pure data-parallel over batch B=8 across the 8 NeuronCores
(one batch element per core, weights replicated). No collectives.

Structure (derived from trace analysis; exp on the Activation engine is
the ~300us roofline, so the schedule exists to keep it saturated):
  1. QKV stage-1 matmuls run from HOST-pre-split fp8 operands: each of
     z, Wq/Wk (x16) and Wv (x16) is split into e4m3 value + e4m3
     residual on the host (the x16 pre-scale keeps K/Q activations
     ~N(0,16), inside IEEE-e4m3 range, and the residuals normal).  The
     three product terms a8*b8 + ar*b8 + a8*br are computed by 9
     DoubleRow fp8 matmuls over c-block PAIRS = 0.75x the PE cost of
     the bf16 6-chain, with no device-side casts and the same DMA
     bytes.  The x16 scales fold into the exp scale and the softmax
     reciprocal for free.
  2. Scores keep the fp8 DoubleRow stack trick ([q8;qr] x [k8;k8|kr;0])
     from the earlier kernel; K/Q stacks scatter at full-S width (10
     DMAs per head pair instead of 20).
  3. The softmax divide of each attention block is carried into the
     NEXT block (emitted after a few of its exps are banked), so its
     DVE-latency never stalls the exp pipeline; cross-branch blocks
     carry at kt6, self at kt2 (before the first pending-proj filler
     pop reads attT).
  4. proj is split into 8 per-(row-block, half) thunks consumed as
     in-block fillers, shrinking the end-of-kernel tail.
  5. Warmup: only the t0 weight slices + z chunk 0 load on the critical
     SP queue, a ~3us PE spin brings the tensor engine to full p-state
     before the first K/Q chains, the Exp table preloads at t=0, and the
     phase-A V-tile copies run on the (otherwise idle until the first
     exp) Activation engine so the PE vps chains run back-to-back, as
     do the t0 fp8 value-casts (the residual sub then self-corrects any
     rounding difference), halving the serial cast chain in front of
     the first stack scatters.
Cost-model: 385.6us vs 392.8us for the previous kernel (ACT busy 300us,
PE 300us, ~41us warmup + ~24us tail remain).
"""

import numpy as np

try:
    import concourse.bass as bass  # noqa: F401
except ImportError:
    import sys

    sys.path.insert(0, "/opt/trn_rl_repo")

import ml_dtypes
from contextlib import ExitStack

import concourse.bass as bass
import concourse.tile as tile
from concourse import bacc, bass_utils, mybir

BF = mybir.dt.bfloat16
F8 = mybir.dt.float8e4
F32 = mybir.dt.float32
EXP = mybir.ActivationFunctionType.Exp
DR = mybir.MatmulPerfMode.DoubleRow

B = 8
SW = 16.0  # host-side scale on Wq/Wk/Wv before the fp8 split (keeps
# K/Q activations ~N(0,16): well inside IEEE-e4m3 range, residuals normal)
N_FULL = 1024
L_FULL = 1024
C_FULL = 768
H_FULL = 12
DH = 64


def build_nc(C=C_FULL, N=N_FULL, L=L_FULL, qw=512, ablate=(), small_out=False):
    S = N + L
    CT = C // 128
    NKT = S // 128
    NKT_SELF = N // 128
    CH = C // 2
    assert CH <= 512 and qw % 128 == 0 and N % qw == 0 and L % qw == 0
    scale = (DH ** -0.5) / (SW * SW)

    nc = bacc.Bacc("TRN2", target_bir_lowering=False, debug=False)
    z8_d = nc.dram_tensor("z8", [C, S], F8, kind="ExternalInput")
    zr_d = nc.dram_tensor("zr", [C, S], F8, kind="ExternalInput")
    wkq8_d = nc.dram_tensor("wkq8", [C, 2 * C], F8, kind="ExternalInput")
    wkqr_d = nc.dram_tensor("wkqr", [C, 2 * C], F8, kind="ExternalInput")
    wqv8_d = nc.dram_tensor("wqv8", [C, C], F8, kind="ExternalInput")
    wqvr_d = nc.dram_tensor("wqvr", [C, C], F8, kind="ExternalInput")
    pw_d = nc.dram_tensor("proj_wt", [C, C], BF, kind="ExternalInput")
    pb_d = nc.dram_tensor("proj_b", [1, C], F32, kind="ExternalInput")
    on = 128 if small_out else N
    ol = 128 if small_out else L
    xo_d = nc.dram_tensor("x_out", [on, C], F32, kind="ExternalOutput")
    yo_d = nc.dram_tensor("y_out", [ol, C], F32, kind="ExternalOutput")

    with tile.TileContext(nc) as tc, ExitStack() as ctx:
        zt_p = ctx.enter_context(tc.tile_pool(name="zt", bufs=1))
        wq_p = ctx.enter_context(tc.tile_pool(name="wq", bufs=1))
        wkq_p = ctx.enter_context(tc.tile_pool(name="wkq", bufs=1))
        qs_p = ctx.enter_context(tc.tile_pool(name="qstk", bufs=2 * CT))
        ks_p = ctx.enter_context(tc.tile_pool(name="kstk", bufs=2 * CT))
        s8_p = ctx.enter_context(tc.tile_pool(name="s8", bufs=2))
        v_p = ctx.enter_context(tc.tile_pool(name="v", bufs=NKT))
        pw_p = ctx.enter_context(tc.tile_pool(name="pw", bufs=CT))
        misc_p = ctx.enter_context(tc.tile_pool(name="misc", bufs=1))
        p2_p = ctx.enter_context(tc.tile_pool(name="p2", bufs=5))
        att_p = ctx.enter_context(tc.tile_pool(name="attq", bufs=2 * CT))
        rr_p = ctx.enter_context(tc.tile_pool(name="rr", bufs=2))
        out_p = ctx.enter_context(tc.tile_pool(name="osb", bufs=2))
        spsum = ctx.enter_context(tc.tile_pool(name="spsum", bufs=2, space="PSUM"))
        apsum = ctx.enter_context(tc.tile_pool(name="apsum", bufs=4, space="PSUM"))
        dram_p = ctx.enter_context(tc.tile_pool(name="dstage", bufs=2, space="DRAM"))

        # z (fp8 + fp8 residual, host-split) packed as ONE [128, CT*S] tile
        # per part; weights likewise host-split into fp8 pairs, pre-scaled by
        # SW so the residuals stay in e4m3 normal range.  Same total bytes as
        # the old bf16 tiles.
        z8 = zt_p.tile([128, CT * S], F8, tag="z8", name="z8_all")
        zr = zt_p.tile([128, CT * S], F8, tag="zr", name="zr_all")
        z83 = z8[:].rearrange("p (c s) -> p c s", c=CT)
        zr3 = zr[:].rearrange("p (c s) -> p c s", c=CT)
        wqv8 = wq_p.tile([128, CT * C], F8, tag="wqv8", name="wqv8")
        wqvr = wq_p.tile([128, CT * C], F8, tag="wqvr", name="wqvr")
        wqv83 = wqv8[:].rearrange("p (c j) -> p c j", c=CT)
        wqvr3 = wqvr[:].rearrange("p (c j) -> p c j", c=CT)
        wkq8 = wkq_p.tile([128, CT * 2 * C], F8, tag="wkq8", name="wkq8")
        wkqr = wkq_p.tile([128, CT * 2 * C], F8, tag="wkqr", name="wkqr")
        wkq83 = wkq8[:].rearrange("p (c j) -> p c j", c=CT)
        wkqr3 = wkqr[:].rearrange("p (c j) -> p c j", c=CT)
        def load_w(dst3, dram, jsl):
            nc.sync.dma_start(
                dst3[:, :, jsl],
                dram.ap().rearrange("(c p) j -> p c j", p=128)[:, :, jsl],
            )

        def load_z(dst3, dram, sl):
            nc.sync.dma_start(
                dst3[:, :, sl], dram.ap().rearrange("(c p) s -> p c s", p=128)[:, :, sl]
            )

        # t0 K/Q weight slices first (all that phase A needs), then z, then
        # the rest: keeps the first K/Q chains off the big weight transfers
        ZCH = S // 2
        load_z(z83, z8_d, slice(0, ZCH))
        load_w(wkq83, wkq8_d, slice(C, C + 128))
        load_w(wkqr3, wkqr_d, slice(C, C + 128))
        load_z(zr3, zr_d, slice(0, ZCH))
        load_w(wkq83, wkq8_d, slice(0, 128))
        load_w(wkqr3, wkqr_d, slice(0, 128))
        load_z(z83, z8_d, slice(ZCH, S))
        load_z(zr3, zr_d, slice(ZCH, S))
        nc.sync.dma_start(wqv83, wqv8_d.ap().rearrange("(c p) j -> p c j", p=128))
        nc.sync.dma_start(wqvr3, wqvr_d.ap().rearrange("(c p) j -> p c j", p=128))
        load_w(wkq83, wkq8_d, slice(C + 128, 2 * C))
        load_w(wkqr3, wkqr_d, slice(C + 128, 2 * C))
        load_w(wkq83, wkq8_d, slice(128, C))
        load_w(wkqr3, wkqr_d, slice(128, C))

        # ps += sum_c (a8+ar)[c]^T b8[c] + a8[c]^T br[c] over c-block PAIRS
        # via DoubleRow fp8 matmuls: 9 matmuls at 0.5 cyc/row instead of the
        # 6-chain bf16 version (0.75x PE cost), dropping only the O(eps^2)
        # ar*br term.
        def dr_chain(ps, a8, ar, b8, br, a_sl, b_sl):
            steps = []
            for c0 in range(0, CT, 2):
                steps.append((a8[:, c0 : c0 + 2, a_sl], b8[:, c0 : c0 + 2, b_sl]))
            for c0 in range(0, CT, 2):
                steps.append((ar[:, c0 : c0 + 2, a_sl], b8[:, c0 : c0 + 2, b_sl]))
            for c0 in range(0, CT, 2):
                steps.append((a8[:, c0 : c0 + 2, a_sl], br[:, c0 : c0 + 2, b_sl]))
            for i, (sa, sb) in enumerate(steps):
                nc.tensor.matmul(
                    ps, sa, sb, start=(i == 0), stop=(i == len(steps) - 1),
                    perf_mode=DR,
                )
        pw = []
        for c in range(CT):
            p1 = pw_p.tile([128, C], BF, tag="pw")
            nc.sync.dma_start(p1[:], pw_d.ap()[c * 128 : (c + 1) * 128, :])
            pw.append(p1)
        bias = misc_p.tile([128, C], F32, tag="bias")
        nc.sync.dma_start(bias[:], pb_d.ap().to_broadcast((128, C)))
        ones = misc_p.tile([128, 64], BF, tag="ones")
        nc.vector.memset(ones[:], 1.0 / SW)
        # spin the PE for ~3us so it reaches full p-state before the first
        # K/Q chains (a cold PE runs them at 1.5-2x cycle time)
        warm_ps = spsum.tile([128, 2 * qw], F32, tag="s2", name="warmps")
        for i in range(40):
            nc.tensor.matmul(
                warm_ps[0:64, 0:64], ones[:, 0:64], ones[:, 0:64],
                start=True, stop=True, skip_group_check=True,
            )

        H = C // DH
        HH = H // 2
        v_sb = [v_p.tile([128, H * 65], BF, tag="v", name=f"v{i}") for i in range(NKT)]
        v_emitted = set()

        def emit_v(st, use_act=False):
            if st in v_emitted:
                return
            v_emitted.add(st)
            vh3 = v_sb[st][:].rearrange("p (h e) -> p h e", e=65)
            nc.vector.memset(vh3[:, :, 64:65], 1.0)
            for vn in range(2):
                ps = apsum.tile([128, CH], F32, tag="acc", name=f"vps{st}_{vn}")
                dr_chain(
                    ps[:], z83, zr3, wqv83, wqvr3,
                    slice(st * 128, (st + 1) * 128),
                    slice(vn * CH, (vn + 1) * CH),
                )
                if use_act:
                    # warmup only: the ACT engine is idle until the first exp,
                    # and moving these copies there lets the PE vps chains run
                    # back-to-back instead of pacing on the DVE copy pipeline
                    nc.scalar.activation(
                        vh3[:, vn * HH : (vn + 1) * HH, 0:64],
                        ps[:].rearrange("p (h e) -> p h e", e=64),
                        mybir.ActivationFunctionType.Copy,
                    )
                else:
                    nc.vector.tensor_copy(
                        vh3[:, vn * HH : (vn + 1) * HH, 0:64],
                        ps[:].rearrange("p (h e) -> p h e", e=64),
                    )

        # Per head h:
        #   qstk[h] [128, S] fp8: rows 0:64 = q8, rows 64:128 = qr.
        #   kstk[h] [128, 2, S] fp8 (DoubleRow subtile dim): sub0 = [k8; k8],
        #     sub1 = [kr; 0].
        # Score matmul per (h, kt): DoubleRow with rhs = qstk broadcast over
        # the subtile dim gives q8k8 + qr k8 + q8 kr.
        qstk = [qs_p.tile([128, S], F8, tag="qstk", name=f"qstk{h}") for h in range(H)]
        kstk = [
            ks_p.tile([128, 2 * S], F8, tag="kstk", name=f"kstk{h}") for h in range(H)
        ]
        kv3 = [kstk[h][:].rearrange("p (i s) -> p i s", i=2) for h in range(H)]
        for h in range(H):
            nc.gpsimd.memset(kv3[h][64:128, 1:2, :], 0.0)
        kq_scratch = {}

        def emit_kq_chain(t, n, is_k, dbase):
            ps = apsum.tile([128, 512], F32, tag="acc", name=f"kq{t}_{n}_{dbase}")
            dr_chain(
                ps[:], wkq83, wkqr3, z83, zr3,
                slice(dbase + t * 128, dbase + (t + 1) * 128),
                slice(n * 512, (n + 1) * 512),
            )
            key = (t, is_k)
            if key not in kq_scratch:
                kq_scratch[key] = (
                    s8_p.tile([128, S], F8, tag="s8", name=f"s8_{t}{dbase}"),
                    s8_p.tile([128, S], F8, tag="r8", name=f"r8_{t}{dbase}"),
                )
            s8, r8 = kq_scratch[key]
            sl = slice(n * 512, (n + 1) * 512)
            with nc.allow_low_precision(reason="fp8 stacks with residual compensation"):
                if t == 0:
                    # warmup: the fp8 value-cast rides the (idle until the
                    # first exp) ACT engine so the DVE only carries the
                    # residual subs - halves the serial cast chain in front
                    # of the first scatters
                    nc.scalar.activation(
                        s8[:, sl], ps[:], mybir.ActivationFunctionType.Copy
                    )
                else:
                    nc.vector.tensor_copy(s8[:, sl], ps[:])
                nc.vector.tensor_sub(r8[:, sl], ps[:], s8[:, sl])
            if n == S // 512 - 1:
                # scatter the full stack in one go: cross-branch queries need
                # the second half immediately anyway, and full-S DMAs halve
                # the per-stack SP/HWDGE issue cost (10 instead of 20 slots)
                del kq_scratch[key]
                a, b = 2 * t, 2 * t + 1
                hs = slice(0, S)
                if is_k:
                    nc.sync.dma_start(kv3[a][0:64, 0:1, hs], s8[0:64, hs])
                    nc.sync.dma_start(kv3[a][64:128, 0:1, hs], s8[0:64, hs])
                    nc.sync.dma_start(kv3[a][0:64, 1:2, hs], r8[0:64, hs])
                    nc.sync.dma_start(kv3[b][0:64, 0:1, hs], s8[64:128, hs])
                    nc.sync.dma_start(kv3[b][64:128, 0:1, hs], s8[64:128, hs])
                    nc.sync.dma_start(kv3[b][0:64, 1:2, hs], r8[64:128, hs])
                else:
                    nc.sync.dma_start(qstk[a][0:64, hs], s8[0:64, hs])
                    nc.sync.dma_start(qstk[a][64:128, hs], r8[0:64, hs])
                    nc.sync.dma_start(qstk[b][0:64, hs], s8[64:128, hs])
                    nc.sync.dma_start(qstk[b][64:128, hs], r8[64:128, hs])

        def kq_chain_thunks(t):
            return [
                (lambda t=t, n=n, is_k=is_k, dbase=dbase: emit_kq_chain(t, n, is_k, dbase))
                for n in range(S // 512)
                for is_k, dbase in ((True, C), (False, 0))
            ]

        def emit_kq(t):
            for th in kq_chain_thunks(t):
                th()

        def emit_attn_hp(branch, qt2, hp, attT, filler=(), prev_div=None):
            filler = list(filler)
            nkt = NKT_SELF if branch == 0 else NKT
            qbase = 0 if branch == 0 else N
            qoff = qbase + qt2 * qw
            accA = apsum.tile([128, qw], F32, tag="acc", name=f"accA{branch}{qt2}{hp}")
            accB = apsum.tile([128, qw], F32, tag="acc", name=f"accB{branch}{qt2}{hp}")
            p2s = []
            for kt in range(nkt):
                s2 = spsum.tile([128, 2 * qw], F32, tag="s2", name=f"s2_{branch}{qt2}{hp}{kt}")
                for hh in range(2):
                    h = 2 * hp + hh
                    q_ap = (
                        qstk[h][:, qoff : qoff + qw]
                        .rearrange("p (i s) -> p i s", i=1)
                        .broadcast_to((128, 2, qw))
                    )
                    nc.tensor.matmul(
                        s2[:, hh * qw : (hh + 1) * qw],
                        kv3[h][:, :, kt * 128 : (kt + 1) * 128],
                        q_ap,
                        start=True,
                        stop=True,
                        perf_mode=DR,
                    )
                p2 = p2_p.tile([128, 2 * qw], BF, tag="p2", name=f"p2_{branch}{qt2}{hp}{kt}")
                nc.scalar.activation(p2[:], s2[:], EXP, scale=scale)
                p2s.append(p2)
                if kt == (2 if branch == 0 else 6) and prev_div is not None:
                    # previous block's divisor tail, emitted once the exp
                    # pipeline has a few tiles banked (absorbs the ~1.4us
                    # DVE-recip latency); in self blocks it must land before
                    # the first pending-proj filler pop reads attT
                    prev_div()
                if filler and ((branch == 0 and kt >= 5) or kt % 4 == 3):
                    filler.pop(0)()
            for kt in range(nkt):
                emit_v(kt)
                first = kt == 0
                last = kt == nkt - 1
                nc.tensor.matmul(
                    accA[0:65, :],
                    v_sb[kt][:, (2 * hp) * 65 : (2 * hp) * 65 + 65],
                    p2s[kt][:, 0:qw],
                    start=first,
                    stop=last,
                    skip_group_check=True,
                )
                nc.tensor.matmul(
                    accB[0:65, :],
                    v_sb[kt][:, (2 * hp + 1) * 65 : (2 * hp + 1) * 65 + 65],
                    p2s[kt][:, qw : 2 * qw],
                    start=first,
                    stop=last,
                    skip_group_check=True,
                )
            while filler:
                filler.pop(0)()

            def div():
                with nc.allow_low_precision(reason="softmax divisor in bf16"):
                    rr = rr_p.tile([128, qw], BF, tag="rr", name=f"rrA{branch}{qt2}{hp}")
                    nc.vector.reciprocal(rr[64:65, :], accA[64:65, :])
                    rr2 = rr_p.tile([128, qw], BF, tag="rr", name=f"rrB{branch}{qt2}{hp}")
                    nc.vector.reciprocal(rr2[64:65, :], accB[64:65, :])
                if branch == 0 and hp == CT - 1:
                    # final block: broadcast the reciprocal with a K=1 PE
                    # outer product instead of the DRAM roundtrip - the PE is
                    # otherwise idle here (tail), this shaves the ~7us serial
                    # DMA chain AND keeps the PE p-state warm for the final
                    # proj chains (ones holds 1/SW, undoing the V scale)
                    bcA = apsum.tile([128, qw], F32, tag="acc", name=f"bcAfin{branch}{qt2}{hp}")
                    nc.tensor.matmul(
                        bcA[0:64, :], ones[64:65, :], rr[64:65, :],
                        start=True, stop=True,
                    )
                    bcB = apsum.tile([128, qw], F32, tag="acc", name=f"bcBfin{branch}{qt2}{hp}")
                    nc.tensor.matmul(
                        bcB[0:64, :], ones[64:65, :], rr2[64:65, :],
                        start=True, stop=True,
                    )
                    numA = rr_p.tile([64, qw], BF, tag="numA", name=f"numAfin{branch}{qt2}{hp}")
                    numB = rr_p.tile([64, qw], BF, tag="numB", name=f"numBfin{branch}{qt2}{hp}")
                    tmpB = rr_p.tile([64, qw], BF, tag="tmpB", name=f"tmpBfin{branch}{qt2}{hp}")
                    with nc.allow_low_precision(reason="softmax divide"):
                        nc.vector.tensor_copy(numA[:], accA[0:64, :])
                        nc.vector.tensor_copy(numB[:], accB[0:64, :])
                        nc.vector.tensor_mul(attT[hp][0:64, :], numA[:], bcA[0:64, :])
                        nc.vector.tensor_mul(tmpB[:], numB[:], bcB[0:64, :])
                    nc.sync.dma_start(attT[hp][64:128, :], tmpB[:])
                    return
                # broadcast the per-query reciprocal across 64 partitions with
                # a K=1 PE outer product (ones columns x recip row) instead of
                # the old DRAM roundtrip (2 stores + 2 broadcast loads per
                # block, which saturated the SP DMA queue); the 1/SW in `ones`
                # undoes the host-side V weight scale.
                numA = rr_p.tile([64, qw], BF, tag="numA", name=f"numA{branch}{qt2}{hp}")
                numB = rr_p.tile([64, qw], BF, tag="numB", name=f"numB{branch}{qt2}{hp}")
                tmpB = rr_p.tile([64, qw], BF, tag="tmpB", name=f"tmpB{branch}{qt2}{hp}")
                with nc.allow_low_precision(reason="softmax divide"):
                    nc.vector.tensor_scalar_mul(numA[:], accA[0:64, :], 1.0 / SW)
                    nc.vector.tensor_scalar_mul(numB[:], accB[0:64, :], 1.0 / SW)
                rs_d = dram_p.tile([2, qw], BF, tag="rsd", name=f"rsd{branch}{qt2}{hp}")
                nc.sync.dma_start(rs_d[0:1, :], rr[64:65, :])
                nc.sync.dma_start(rs_d[1:2, :], rr2[64:65, :])
                rbA = rr_p.tile([64, qw], BF, tag="rbA", name=f"rbA{branch}{qt2}{hp}")
                nc.sync.dma_start(rbA[:], rs_d[0:1, :].to_broadcast((64, qw)))
                rbB = rr_p.tile([64, qw], BF, tag="rbB", name=f"rbB{branch}{qt2}{hp}")
                nc.sync.dma_start(rbB[:], rs_d[1:2, :].to_broadcast((64, qw)))
                nc.vector.tensor_mul(attT[hp][0:64, :], numA[:], rbA[:])
                nc.vector.tensor_mul(tmpB[:], numB[:], rbB[:])
                nc.sync.dma_start(attT[hp][64:128, :], tmpB[:])

            return div

        def proj_thunks(branch, qt2, attT):
            out_d = xo_d if branch == 0 else yo_d
            osb_state = {}

            def th(lt, half):
                if lt not in osb_state:
                    osb_state[lt] = out_p.tile(
                        [128, C], F32, tag="osb", name=f"osb{branch}{qt2}{lt}"
                    )
                osb = osb_state[lt]
                pp = apsum.tile(
                    [128, CH], F32, tag="acc", name=f"pp{branch}{qt2}{lt}{half}"
                )
                for ct in range(CT):
                    nc.tensor.matmul(
                        pp[:],
                        attT[ct][:, lt * 128 : (lt + 1) * 128],
                        pw[ct][:, half * CH : (half + 1) * CH],
                        start=(ct == 0),
                        stop=(ct == CT - 1),
                    )
                nc.vector.tensor_add(
                    osb[:, half * CH : (half + 1) * CH],
                    pp[:],
                    bias[:, half * CH : (half + 1) * CH],
                )
                if half == 1:
                    row0 = qt2 * qw + lt * 128
                    if small_out:
                        if row0 == 0:
                            nc.sync.dma_start(out_d.ap()[0:128, :], osb[:])
                    else:
                        nc.sync.dma_start(out_d.ap()[row0 : row0 + 128, :], osb[:])

            return [
                (lambda lt=lt, half=half: th(lt, half))
                for lt in range(qw // 128)
                for half in range(2)
            ]

        def emit_proj(branch, qt2, attT):
            for th in proj_thunks(branch, qt2, attT):
                th()

        def alloc_attT(tagix):
            return [
                att_p.tile([128, qw], BF, tag="attT", name=f"attT{tagix}_{i}")
                for i in range(CT)
            ]

        # phase A: the full t0 K/Q stacks (cross scores need the half-1
        # query columns immediately) and the first half of the V tiles; the
        # rest arrive as in-block fillers.
        emit_kq(0)
        for st in range(NKT):
            emit_v(st, use_act=True)

        nq_self = N // qw
        nq_cross = L // qw
        prev_div = None
        attT_cross = [alloc_attT(f"c{q}") for q in range(nq_cross)]
        for hp in range(CT):
            thunks = kq_chain_thunks(hp + 1) if hp + 1 < CT else []
            fill0 = thunks[: len(thunks) // 2] if nq_cross > 1 else thunks
            rest = thunks[len(fill0) :]
            prev_div = emit_attn_hp(
                1, 0, hp, attT_cross[0], filler=fill0, prev_div=prev_div
            )
            for q in range(1, nq_cross):
                prev_div = emit_attn_hp(
                    1, q, hp, attT_cross[q], filler=rest, prev_div=prev_div
                )
                rest = []
            for th in rest:
                th()
        pending = proj_thunks(1, 0, attT_cross[0]) + proj_thunks(1, 1, attT_cross[1])
        attT_self = [alloc_attT(f"s{q}") for q in range(nq_self)]
        for qt2 in range(nq_self):
            for hp in range(CT):
                fill = pending[:8]
                pending = pending[8:]
                prev_div = emit_attn_hp(
                    0, qt2, hp, attT_self[qt2], filler=fill, prev_div=prev_div
                )
            pending = pending + proj_thunks(0, qt2, attT_self[qt2])
        prev_div()
        for th in pending:
            th()

    nc.compile()
    return nc


def _split_fp8(a):
    a8 = a.astype(ml_dtypes.float8_e4m3)
    ar = (a - a8.astype(np.float32)).astype(ml_dtypes.float8_e4m3)
    return a8, ar


def prep_weights(qkv_w, proj_w, proj_b):
    C = C_FULL
    wkq8, wkqr = _split_fp8(np.ascontiguousarray(qkv_w[: 2 * C].T) * SW)
    wqv8, wqvr = _split_fp8(np.ascontiguousarray(qkv_w[2 * C :].T) * SW)
    return {
        "wkq8": wkq8,
        "wkqr": wkqr,
        "wqv8": wqv8,
        "wqvr": wqvr,
        "proj_wt": np.ascontiguousarray(proj_w.T).astype(ml_dtypes.bfloat16),
        "proj_b": proj_b.reshape(1, -1).astype(np.float32),
    }


def _prep_core_inputs(xb, yb, wmap):
    z = np.concatenate([xb, yb], axis=0)
    z8, zr = _split_fp8(np.ascontiguousarray(z.T).astype(np.float32))
    d = {"z8": z8, "zr": zr}
    d.update(wmap)
    return d


_NC_CACHE = {}


def kernel(x, y, qkv_w, proj_w, proj_b):
    x = np.asarray(x, dtype=np.float32)
    y = np.asarray(y, dtype=np.float32)
    qkv_w = np.asarray(qkv_w, dtype=np.float32)
    proj_w = np.asarray(proj_w, dtype=np.float32)
    proj_b = np.asarray(proj_b, dtype=np.float32)

    wmap = prep_weights(qkv_w, proj_w, proj_b)
    in_maps = [_prep_core_inputs(x[b], y[b], wmap) for b in range(x.shape[0])]
    if "nc" not in _NC_CACHE:
        _NC_CACHE["nc"] = build_nc()
    nc = _NC_CACHE["nc"]
    res = bass_utils.run_bass_kernel_spmd(nc, in_maps, core_ids=list(range(len(in_maps))))
    x_out = np.stack([res.results[b]["x_out"] for b in range(len(in_maps))])
    y_out = np.stack([res.results[b]["y_out"] for b in range(len(in_maps))])
    return (x_out, y_out)

